# revision 34
# baseline (speedup 1.0000x reference)
"""DGCNN (dynamic edge conv x2 + classifier) Trainium2 Bass kernel.

Sharding: data-parallel over the 8 point clouds -> 8 NeuronCores.

v2 pipeline (per 128-point tile):
  * kNN scores via one augmented matmul (as v1): psc = [x,1]^T [-2x; |x|^2],
    ACT negate-copy (-psc - s_i) -> negS fp32 in SBUF.
  * top-24 via hierarchical DVE selection: 8 group max8/max_index over
    256-wide column groups -> 64 exact candidates; candidates packed into
    fp32 keys (12-bit quantized value * 2048 + (2047 - global_idx)) so the
    final top-24 merge needs only max8/match_replace and the winning
    indices fall out of the key bits with 3 tiny tensor_scalar ops.
  * neighbor features fetched with ONE gpsimd dma_gather(transpose=True)
    per tile straight from DRAM v-rows (bf16, 256B each) into channel-major
    [128ch, 20*128] SBUF layout - no PE transposes, no per-slot DMAs.
  * z1 = relu(a_i + v_j) as a single bf16 DVE add (2x mode) + ACT relu.
  * edge-MLP layers 2/3 in bf16; layer-3 output max-reduced over K=20
    straight out of 2-bank PSUM tiles (k-contiguous 320-column blocks).
"""

import os
import sys
import numpy as np

for _p in ("/opt/trn_rl_repo",):
    if _p not in sys.path:
        sys.path.insert(0, _p)

N = 2048          # points per cloud
NCLOUD = 8
P = 128           # partition tile
NT = N // P       # 16 row tiles
KNN = 20
KSEL = 24         # 3 rounds x 8
G = 4             # score groups per row (hierarchical top-k)
GW = N // G       # 512 group width
NCAND = G * 8     # 32 merge candidates
NEG_BIG = -3.0e38
NUM_CLASSES = 40
F23 = 8388608.0   # 2^23
RNGQ = 128.0      # d2 quantization range for the candidate merge keys
# 11-bit quantized value in a x4096 granule; quantization happens via fp32
# ulp rounding at 2^35 magnitude (single tensor_scalar), index payload
# 2048-jg (never 0, so the floor-extraction add at 2^35-2048 has no
# round-half tie).
SQ4 = (2047.0 / RNGQ) * 4096.0
B35 = (2047.0 + F23) * 4096.0    # 2^35 + 2^23 - 2^12, exactly representable
T35 = 34359738368.0              # 2^35
W2B = T35 - 2048.0               # floor-extraction bias

MM_FAST_MLP = True   # f32r fast path for classifier matmuls

DEBUG = os.environ.get("BASSK_DEBUG") == "1"

_PROGRAM_CACHE = {}


def _build_program():
    import concourse.bass as bass
    import concourse.bacc as bacc
    import concourse.tile as tile
    from concourse import mybir

    f32 = mybir.dt.float32
    f32r = mybir.dt.float32r
    bf16 = mybir.dt.bfloat16
    u16 = mybir.dt.uint16
    i16 = mybir.dt.int16
    AX = mybir.AxisListType
    OP = mybir.AluOpType
    ACT = mybir.ActivationFunctionType

    def mm(ap):
        return ap.bitcast(f32r) if MM_FAST_MLP else ap

    mmo = mm

    nc = bacc.Bacc("TRN2", target_bir_lowering=False, debug=False,
                   num_swdge_queues=4)

    # ---------------- I/O ----------------
    def din(name, shape):
        return nc.dram_tensor(name, list(shape), f32, kind="ExternalInput").ap()

    pos = din("pos", [N, 3])
    c1w1 = din("c1w1", [6, 64]);   c1b1 = din("c1b1", [64])
    c1w2 = din("c1w2", [64, 64]);  c1b2 = din("c1b2", [64])
    c1w3 = din("c1w3", [64, 64]);  c1b3 = din("c1b3", [64])
    c2w1 = din("c2w1", [128, 128]); c2b1 = din("c2b1", [128])
    c2w2 = din("c2w2", [128, 128]); c2b2 = din("c2b2", [128])
    c2w3 = din("c2w3", [128, 256]); c2b3 = din("c2b3", [256])
    l0w = din("l0w", [256, 512]);  l0b = din("l0b", [512])
    l1w = din("l1w", [512, 256]);  l1b = din("l1b", [256])
    l2w = din("l2w", [256, 256]);  l2b = din("l2b", [256])
    l3w = din("l3w", [256, NUM_CLASSES]); l3b = din("l3b", [NUM_CLASSES])
    ident = din("ident", [128, 128])

    out = nc.dram_tensor("out", [1, NUM_CLASSES], f32, kind="ExternalOutput").ap()

    dbg = {}
    if DEBUG:
        for nm, shp, dt_ in [
            ("d_negS0", [128, N], f32), ("d_candv0", [128, NCAND], f32),
            ("d_key0", [128, NCAND], f32), ("d_j24_0", [128, KSEL], f32),
            ("d_idxw0", [128, 8 * KNN], f32),
            ("d_gath0", [128, KNN * 128], f32),
            ("d_z1T0", [128, KNN * 128], f32),
            ("d_x1T", [64, N], f32),
            ("d_x2Ta", [128, N], f32), ("d_x2Tb", [128, N], f32),
        ]:
            dbg[nm] = nc.dram_tensor(nm, shp, dt_, kind="ExternalOutput").ap()

    with tile.TileContext(nc) as tc:
        from contextlib import ExitStack

        ctx = ExitStack()
        g = ctx.enter_context(tc.tile_pool(name="g", bufs=1))          # persistent
        dpool = ctx.enter_context(tc.tile_pool(name="dram", bufs=1, space="DRAM"))

        ident_sb = g.tile([128, 128], f32)
        nc.sync.dma_start(ident_sb[:], ident[:, :])

        A1 = g.tile([3, N], f32r)      # x^T (f32r for fast score matmuls)
        B1 = g.tile([3, N], f32r)      # -2 x^T
        s1r = g.tile([1, N], f32r)     # |x|^2 row
        A2 = g.tile([64, N], f32r)     # x1^T
        B2 = g.tile([64, N], f32r)     # -2 x1^T
        s2r = g.tile([1, N], f32r)     # |x1|^2 row
        onesr = g.tile([1, N], f32r)   # ones row (rank-1 score lhsT)
        a1T = g.tile([64, N], f32)     # u1 - v1 + b1  (channel-major)
        a1Tb = g.tile([64, N], bf16)
        a2T = g.tile([128, N], bf16)
        x2Ta = g.tile([128, N], f32)   # conv2 out ch 0:128
        x2Tb = g.tile([128, N], f32)   # conv2 out ch 128:256
        nscol1 = g.tile([128, NT], f32)  # -s_i per tile column
        nscol2 = g.tile([128, NT], f32)
        vscratch = g.tile([128, N], f32, name="vscratch")

        # candidate-key merge constant: col j -> 2048 - GW*(j//8)
        C32 = g.tile([128, NCAND], f32, name="C32")
        for gg in range(G):
            nc.vector.memset(C32[:, gg * 8:(gg + 1) * 8], 2048.0 - GW * gg)

        v1d = dpool.tile([N, 128], bf16, name="v1d")
        v2d = dpool.tile([N, 128], bf16, name="v2d")

        wraw = ctx.enter_context(tc.tile_pool(name="wraw", bufs=2))

        def load_w(name, shape, pieces, wdt=f32):
            if wdt != f32:
                raw = wraw.tile(list(shape), f32, name=name + "_raw", tag="wraw")
                for sl, srcap in pieces:
                    nc.sync.dma_start(raw[sl], srcap)
                t = g.tile(list(shape), wdt, name=name)
                nc.scalar.copy(t[:, :], raw[:, :])
            else:
                t = g.tile(list(shape), f32, name=name)
                for sl, srcap in pieces:
                    nc.sync.dma_start(t[sl], srcap)
            return t

        SALL = (slice(None), slice(None))
        w_c1w1a = load_w("w_c1w1a", [3, 64], [(SALL, c1w1[0:3, :])], wdt=f32r)
        w_c1w1b = load_w("w_c1w1b", [3, 64], [(SALL, c1w1[3:6, :])], wdt=f32r)
        w_c2w1a = load_w("w_c2w1a", [64, 128], [(SALL, c2w1[0:64, :])], wdt=f32r)
        w_c2w1b = load_w("w_c2w1b", [64, 128], [(SALL, c2w1[64:128, :])], wdt=f32r)
        w_c1w2 = load_w("w_c1w2", [64, 64], [(SALL, c1w2[:, :])], wdt=bf16)
        w_c1w3 = load_w("w_c1w3", [64, 64], [(SALL, c1w3[:, :])], wdt=bf16)
        w_c2w2 = load_w("w_c2w2", [128, 128], [(SALL, c2w2[:, :])], wdt=bf16)
        w_c2w3 = load_w("w_c2w3", [128, 256], [(SALL, c2w3[:, :])], wdt=bf16)
        wdt_head = f32r if MM_FAST_MLP else f32
        w_l0w = load_w("w_l0w", [128, 1024],
                       [((slice(None), slice(0, 512)), l0w[0:128, :]),
                        ((slice(None), slice(512, 1024)), l0w[128:256, :])],
                       wdt=wdt_head)
        w_l1w = load_w("w_l1w", [128, 1024],
                       [((slice(None), slice(c * 256, (c + 1) * 256)),
                         l1w[c * 128:(c + 1) * 128, :]) for c in range(4)],
                       wdt=wdt_head)
        w_l2w = load_w("w_l2w", [128, 512],
                       [((slice(None), slice(0, 256)), l2w[0:128, :]),
                        ((slice(None), slice(256, 512)), l2w[128:256, :])],
                       wdt=wdt_head)
        w_l3w = load_w("w_l3w", [128, 2 * NUM_CLASSES],
                       [((slice(None), slice(0, NUM_CLASSES)), l3w[0:128, :]),
                        ((slice(None), slice(NUM_CLASSES, 2 * NUM_CLASSES)), l3w[128:256, :])],
                       wdt=wdt_head)

        def col(name, src, n):
            t = g.tile([n, 1], f32, name=name)
            nc.sync.dma_start(t[:, :], src.rearrange("(c o) -> c o", o=1))
            return t

        b_c1b1 = col("b_c1b1", c1b1, 64)
        b_c1b2 = col("b_c1b2", c1b2, 64)
        b_c1b3 = col("b_c1b3", c1b3, 64)
        b_c2b1 = col("b_c2b1", c2b1, 128)
        b_c2b2 = col("b_c2b2", c2b2, 128)
        b_c2b3 = g.tile([128, 2], f32)
        nc.sync.dma_start(b_c2b3[:, 0:1], c2b3.rearrange("(h c o) -> h c o", h=2, o=1)[0])
        nc.sync.dma_start(b_c2b3[:, 1:2], c2b3.rearrange("(h c o) -> h c o", h=2, o=1)[1])
        b_l0b = g.tile([128, 4], f32)
        for t_ in range(4):
            nc.sync.dma_start(b_l0b[:, t_:t_ + 1],
                              l0b.rearrange("(h c o) -> h c o", h=4, o=1)[t_])
        b_l1b = g.tile([128, 2], f32)
        for t_ in range(2):
            nc.sync.dma_start(b_l1b[:, t_:t_ + 1],
                              l1b.rearrange("(h c o) -> h c o", h=2, o=1)[t_])
        b_l2b = g.tile([128, 2], f32)
        for t_ in range(2):
            nc.sync.dma_start(b_l2b[:, t_:t_ + 1],
                              l2b.rearrange("(h c o) -> h c o", h=2, o=1)[t_])
        b_l3b = col("b_l3b", l3b, NUM_CLASSES)

        # ones row (f32r) for the rank-1 |x_j|^2 score matmul
        ones_f = g.tile([1, N], f32, name="ones_f")
        nc.vector.memset(ones_f[:, :], 1.0)
        nc.scalar.copy(onesr[:, :], ones_f[:, :])

        # =============== conv1 prep ===============
        with tc.tile_pool(name="prep", bufs=2) as pp, \
             tc.tile_pool(name="prep_ps", bufs=2, space="PSUM") as ppp:
            scol = g.tile([128, NT], f32, name="scol1_pos")
            for i in range(NT):
                isl = slice(i * P, (i + 1) * P)
                pt = pp.tile([128, 3], f32, name="pt")
                nc.sync.dma_start(pt[:], pos[isl, :])
                sq = pp.tile([128, 3], f32, name="sq")
                nc.scalar.activation(sq[:], pt[:], ACT.Square,
                                     accum_out=scol[:, i:i + 1])
                tp = ppp.tile([3, 128], f32, name="tp", space="PSUM", bufs=1)
                nc.tensor.transpose(tp[:], pt[:], ident_sb[:])
                nc.scalar.copy(A1[0:3, isl], tp[:])
            nc.scalar.mul(nscol1[:, :], scol[:, :], -1.0)
            nc.scalar.mul(B1[0:3, :], A1[0:3, :].bitcast(f32), -2.0)
            stp = ppp.tile([NT, 128], f32, name="stp", space="PSUM", bufs=1)
            nc.tensor.transpose(stp[:], scol[:, :], ident_sb[:])
            srow_sb = pp.tile([NT, 128], f32, name="srow_sb")
            nc.scalar.copy(srow_sb[:, :], stp[:, :])
            s1f = pp.tile([1, N], f32, name="s1f")
            nc.sync.dma_start(
                s1f[:, :].rearrange("o (p n) -> o p n", p=NT), srow_sb[:, :])
            nc.scalar.copy(s1r[:, :], s1f[:, :])

            # u1/v1, a1T, v1 rows -> DRAM (bf16, padded to 128 ch = 256B rows)
            for c in range(4):
                cs = slice(c * 512, (c + 1) * 512)
                pu = ppp.tile([64, 512], f32, name="pu", space="PSUM")
                nc.tensor.matmul(pu[:], w_c1w1a[:, :], A1[0:3, cs])
                nc.scalar.activation(a1T[:, cs], pu[:], ACT.Identity, bias=b_c1b1[:, 0:1])
                pv = ppp.tile([64, 512], f32, name="pv", space="PSUM")
                nc.tensor.matmul(pv[:], w_c1w1b[:, :], A1[0:3, cs])
                nc.scalar.copy(vscratch[0:64, cs], pv[:])
                nc.vector.tensor_sub(a1T[:, cs], a1T[:, cs], pv[:])
            nc.scalar.copy(a1Tb[:, :], a1T[:, :])
            for grp in range(4):
                vstage = pp.tile([128, 512], bf16, name="vstage")
                for m in range(4):
                    i = grp * 4 + m
                    tvp = ppp.tile([128, 64], f32, name="tvp", space="PSUM")
                    nc.tensor.transpose(tvp[:], vscratch[0:64, i * P:(i + 1) * P],
                                        ident_sb[0:64, 0:64])
                    nc.vector.memset(vstage[:, m * 128 + 64:(m + 1) * 128], 0.0)
                    nc.vector.tensor_copy(vstage[:, m * 128:m * 128 + 64], tvp[:])
                nc.sync.dma_start(
                    v1d[:, :].rearrange("(g m r) ch -> g r m ch", g=4, m=4)[grp],
                    vstage[:, :])

        # =============== edge-conv block (shared structure) ===============
        gsem = [nc.alloc_semaphore(f"gsem{q}") for q in range(4)]
        gcnt = [0, 0, 0, 0]

        def edge_conv(conv, sp, spp, dsp):
            if conv == 1:
                H, CON = 64, 3
                Asb, Bsb, srow, aTb, vd, nscol = A1, B1, s1r, a1Tb, v1d, nscol1
                wl2, wl3 = w_c1w2, w_c1w3
                bl2, bl3 = b_c1b2, b_c1b3
                nhalf = 1
            else:
                H, CON = 128, 64
                Asb, Bsb, srow, aTb, vd, nscol = A2, B2, s2r, a2T, v2d, nscol2
                wl2, wl3 = w_c2w2, w_c2w3
                bl2, bl3 = b_c2b2, b_c2b3
                nhalf = 2

            state = {}

            def stage_scores(i):
                isl = slice(i * P, (i + 1) * P)
                negS = sp.tile([128, N], f32, name="negS", tag="negS", bufs=3)
                for c in range(4):
                    cs = slice(c * 512, (c + 1) * 512)
                    psc = spp.tile([128, 512], f32, name="psc", tag="psc", bufs=2)
                    nc.tensor.matmul(psc[:, :], Asb[0:CON, isl], Bsb[0:CON, cs],
                                     start=True, stop=False)
                    nc.tensor.matmul(psc[:, :], onesr[0:1, isl], srow[0:1, cs],
                                     start=False, stop=True)
                    nc.scalar.activation(negS[:, cs], psc[:, :], ACT.Identity,
                                         bias=nscol[:, i:i + 1], scale=-1.0)
                state[i] = {"negS": negS}
                if DEBUG and i == 0 and conv == 1:
                    nc.sync.dma_start(dbg["d_negS0"], negS[:, :])

            def stage_topk(i):
                negS = state[i]["negS"]
                # hierarchical selection: per-group exact top-8 (values+indices)
                candv = sp.tile([128, NCAND], f32, name="candv", tag="candv", bufs=2)
                candi = sp.tile([128, NCAND], u16, name="candi", tag="candi", bufs=2)
                for gg in range(G):
                    nc.vector.max(candv[:, gg * 8:(gg + 1) * 8],
                                  negS[:, gg * GW:(gg + 1) * GW])
                for gg in range(G):
                    nc.vector.max_index(candi[:, gg * 8:(gg + 1) * 8],
                                        candv[:, gg * 8:(gg + 1) * 8],
                                        negS[:, gg * GW:(gg + 1) * GW])
                # pack: key = q11*4096 + 2048 - global_idx, via one ulp-rounded
                # quantize at 2^35 plus a fused un-bias/add chain
                tq = sp.tile([128, NCAND], f32, name="tq", tag="tq")
                nc.vector.tensor_scalar(tq[:, :], candv[:, :], SQ4, B35,
                                        op0=OP.mult, op1=OP.add)
                cif = sp.tile([128, NCAND], f32, name="cif", tag="cif")
                nc.vector.tensor_scalar(cif[:, :], candi[:, :], -1.0, None,
                                        op0=OP.mult)
                yq = sp.tile([128, NCAND], f32, name="yq", tag="yq")
                nc.vector.scalar_tensor_tensor(yq[:, :], tq[:, :], T35, cif[:, :],
                                               op0=OP.subtract, op1=OP.add)
                key = sp.tile([128, NCAND], f32, name="key", tag="key")
                nc.vector.tensor_add(key[:, :], yq[:, :], C32[:, :])
                if DEBUG and i == 0 and conv == 1:
                    nc.sync.dma_start(dbg["d_candv0"], candv[:, :])
                    nc.sync.dma_start(dbg["d_key0"], key[:, :])
                # top-24 merge on packed keys
                vals = sp.tile([128, KSEL], f32, name="vals", tag="vals")
                for r in range(3):
                    rs = slice(r * 8, (r + 1) * 8)
                    nc.vector.max(vals[:, rs], key[:, :])
                    if r < 2:
                        nc.vector.match_replace(key[:, :], vals[:, rs],
                                                key[:, :], NEG_BIG)
                # index extraction from key bits (floor at 2^35 ulp, no ties)
                ut = sp.tile([128, KSEL], f32, name="ut", tag="ut")
                nc.vector.tensor_scalar(ut[:, :], vals[:, :], W2B, T35,
                                        op0=OP.add, op1=OP.subtract)
                j24 = sp.tile([128, KSEL], i16, name="j24", tag="j24")
                nc.vector.scalar_tensor_tensor(j24[:, :], ut[:, :], 2048.0,
                                               vals[:, :], op0=OP.add,
                                               op1=OP.subtract)
                if DEBUG and i == 0 and conv == 1:
                    nc.gpsimd.dma_start(dbg["d_j24_0"], j24[:, :])
                # restride to the wrapped int16 layout dma_gather expects:
                # flat slot s = (n_hi*20 + k)*16 + p  ->  idxw[p, n_hi*20+k],
                # replicated into all 8 16-partition groups.  SBUF APs cannot
                # regroup partitions inside free dims, so the fold bounces
                # through flat DRAM (j24 -> jd -> jw wrapped -> broadcast).
                jd = dsp.tile([128, KSEL], i16, name="jd", tag="jd", bufs=2)
                jw = dsp.tile([16, 8 * KNN], i16, name="jw", tag="jw", bufs=2)
                jw2 = dsp.tile([128, 8 * KNN], i16, name="jw2", tag="jw2", bufs=2)
                idxw = sp.tile([128, 8 * KNN], i16, name="idxw", tag="idxw", bufs=3)
                nc.sync.dma_start(jd[:, :], j24[:, :])
                nc.sync.dma_start(
                    jw[:, :].rearrange("p (h k) -> p h k", k=KNN),
                    jd.rearrange("(h p) k -> p h k", p=16)[:, :, 0:KNN])
                nc.scalar.dma_start(
                    jw2[:, :].rearrange("(g p) c -> g p c", p=16),
                    jw[:, :].rearrange("p (o c) -> o p c", o=1)
                        .to_broadcast([8, 16, 8 * KNN]))
                nc.scalar.dma_start(idxw[:, :], jw2[:, :])
                # channel-major gather of the 20*128 neighbor rows; split into
                # 640-idx chunks (a single 2560-idx transpose gather wedges
                # the device - descriptor-ring scale limit).
                gathT = sp.tile([128, KNN * 128], bf16, name="gath", tag="gath",
                                bufs=3)
                for it in range(4):
                    nc.gpsimd.dma_gather(
                        gathT[:, it * 640:(it + 1) * 640]
                            .rearrange("p (o n) -> p o n", o=1),
                        vd[:, :], idxw[:, it * 40:(it + 1) * 40],
                        num_idxs=640, num_idxs_reg=640,
                        elem_size=128, transpose=True,
                        queue_num=0).then_inc(gsem[0], 16)
                    gcnt[0] += 16
                # Tile's SWDGE completion lanes are not queue-aware; a later
                # gather on another queue can bump the lane a consumer waits
                # on.  Gate consumers on explicit per-queue counts instead.
                state[i].update(gathT=gathT, gtarget=tuple(gcnt))
                if DEBUG and i == 0 and conv == 1:
                    nc.gpsimd.dma_start(dbg["d_idxw0"], idxw[:, :])
                    nc.gpsimd.dma_start(dbg["d_gath0"], gathT[:, :])

            def stage_mlp(i):
                isl = slice(i * P, (i + 1) * P)
                gathT = state[i]["gathT"]
                for q in range(4):
                    nc.vector.wait_ge(gsem[q], state[i]["gtarget"][q])
                # z1 = relu(a_i + v_j), channel-major, bf16
                z1T = sp.tile([H, KNN * 128], bf16, name="z1T", tag="z1T", bufs=2)
                gv = gathT[0:H, :].rearrange("c (a k p) -> c a k p", k=KNN, p=16)
                av = aTb[:, isl].rearrange("c (a o p) -> c a o p", o=1, p=16) \
                                .to_broadcast([H, 8, KNN, 16])
                nc.vector.tensor_add(
                    z1T.rearrange("c (a k p) -> c a k p", k=KNN, p=16), gv, av)
                nc.scalar.activation(z1T[:, :], z1T[:, :], ACT.Relu)
                if DEBUG and i == 0 and conv == 1:
                    nc.gpsimd.dma_start(dbg["d_z1T0"][0:H, :], z1T[:, :])
                # ---- layer 2 ----
                z2T = sp.tile([H, KNN * 128], bf16, name="z2T", tag="z2T", bufs=2)
                for c in range(5):
                    cs = slice(c * 512, (c + 1) * 512)
                    pm = spp.tile([H, 512], f32, name="pm", tag="pm", bufs=2)
                    nc.tensor.matmul(pm[:], wl2[:, :], z1T[:, cs])
                    nc.scalar.activation(z2T[:, cs], pm[:], ACT.Relu,
                                         bias=bl2[:, 0:1])
                # ---- layer 3 + max over K (k-contiguous 320-col blocks) ----
                red = sp.tile([128, 128], f32, name="red", tag="red", bufs=2)
                for h in range(nhalf):
                    wsel = wl3[:, :] if conv == 1 else wl3[:, h * 128:(h + 1) * 128]
                    for t2 in range(4):
                        pl = spp.tile([H, 1024], f32, name="pl", tag="pl", bufs=2)
                        for b2 in range(2):
                            blk = t2 * 2 + b2
                            nc.tensor.matmul(
                                pl[:, b2 * 512:b2 * 512 + 320], wsel,
                                z2T[:, blk * 320:(blk + 1) * 320])
                        rv = pl.rearrange("c (b r) -> c b r", b=2)[:, :, 0:320] \
                               .rearrange("c b (k p) -> c b p k", p=16)
                        nc.vector.tensor_reduce(
                            red[0:H, t2 * 32:(t2 + 1) * 32]
                                .rearrange("c (b p) -> c b p", b=2),
                            rv, axis=AX.X, op=OP.max)
                    if conv == 1:
                        nc.scalar.activation(A2[0:64, isl], red[0:64, :],
                                             ACT.Relu, bias=bl3[:, 0:1])
                    else:
                        dst = x2Ta if h == 0 else x2Tb
                        nc.scalar.activation(mmo(dst[:, isl]), red[:, :],
                                             ACT.Relu, bias=bl3[:, h:h + 1])
                del state[i]

            stage_scores(0)
            stage_topk(0)
            for i in range(NT):
                if i + 1 < NT:
                    stage_scores(i + 1)
                    stage_topk(i + 1)
                stage_mlp(i)

        # =============== conv1 ===============
        with tc.tile_pool(name="c1", bufs=2) as sp, \
             tc.tile_pool(name="c1d", bufs=2, space="DRAM") as dsp, \
             tc.tile_pool(name="c1ps", bufs=2, space="PSUM") as spp:
            edge_conv(1, sp, spp, dsp)
        if DEBUG:
            nc.sync.dma_start(dbg["d_x1T"], A2[0:64, :].bitcast(f32))

        # =============== conv2 prep ===============
        with tc.tile_pool(name="prep2", bufs=2) as pp, \
             tc.tile_pool(name="prep2_ps", bufs=2, space="PSUM") as ppp:
            nc.scalar.activation(vscratch[0:64, :], A2[0:64, :].bitcast(f32),
                                 ACT.Square)
            ones64 = g.tile([64, 1], f32, name="ones64")
            nc.vector.memset(ones64[:, :], 1.0)
            s2tmp = pp.tile([1, N], f32, name="s2tmp")
            for c in range(4):
                cs = slice(c * 512, (c + 1) * 512)
                ps2 = ppp.tile([1, 512], f32, name="ps2", space="PSUM", bufs=1)
                nc.tensor.matmul(ps2[:], ones64[:, :], vscratch[0:64, cs])
                nc.scalar.copy(s2tmp[0:1, cs], ps2[:])
            nc.scalar.copy(s2r[:, :], s2tmp[:, :])
            for i in range(NT):
                isl = slice(i * P, (i + 1) * P)
                tsc = ppp.tile([128, 1], f32, name="tsc", space="PSUM", bufs=1)
                nc.tensor.transpose(tsc[:], s2tmp[0:1, isl], ident_sb[0:1, 0:1])
                nc.scalar.mul(nscol2[:, i:i + 1], tsc[:], -1.0)
            nc.scalar.mul(B2[0:64, :], A2[0:64, :].bitcast(f32), -2.0)
            for c in range(4):
                cs = slice(c * 512, (c + 1) * 512)
                pu = ppp.tile([128, 512], f32, name="pu2", space="PSUM")
                nc.tensor.matmul(pu[:], w_c2w1a[:, :], A2[0:64, cs])
                nc.scalar.activation(a2T[:, cs], pu[:], ACT.Identity, bias=b_c2b1[:, 0:1])
                pv = ppp.tile([128, 512], f32, name="pv2", space="PSUM")
                nc.tensor.matmul(pv[:], w_c2w1b[:, :], A2[0:64, cs])
                nc.scalar.copy(vscratch[:, cs], pv[:])
                nc.vector.tensor_sub(a2T[:, cs], a2T[:, cs], pv[:])
            for grp in range(4):
                vstage = pp.tile([128, 512], bf16, name="vstage2")
                for m in range(4):
                    i = grp * 4 + m
                    tvp = ppp.tile([128, 128], f32, name="tvp2", space="PSUM")
                    nc.tensor.transpose(tvp[:], vscratch[:, i * P:(i + 1) * P],
                                        ident_sb[:, :])
                    nc.vector.tensor_copy(vstage[:, m * 128:(m + 1) * 128], tvp[:])
                nc.sync.dma_start(
                    v2d[:, :].rearrange("(g m r) ch -> g r m ch", g=4, m=4)[grp],
                    vstage[:, :])

        # =============== conv2 ===============
        with tc.tile_pool(name="c2", bufs=2) as sp, \
             tc.tile_pool(name="c2d", bufs=2, space="DRAM") as dsp, \
             tc.tile_pool(name="c2ps", bufs=2, space="PSUM") as spp:
            edge_conv(2, sp, spp, dsp)

        if DEBUG:
            nc.sync.dma_start(dbg["d_x2Ta"], x2Ta[:, :])
            nc.sync.dma_start(dbg["d_x2Tb"], x2Tb[:, :])

        # =============== classifier ===============
        with tc.tile_pool(name="cls", bufs=2) as cp, \
             tc.tile_pool(name="clsps", bufs=2, space="PSUM") as cpp:
            pooled = g.tile([128, 4], f32, name="pooled")
            for t_ in range(4):
                tsl = slice(t_ * 128, (t_ + 1) * 128)
                pool4 = cp.tile([128, 4], f32, name="pool4")
                for c in range(4):
                    cs = slice(c * 512, (c + 1) * 512)
                    ps = cpp.tile([128, 512], f32, name="ps_l0", tag="ps_l0")
                    nc.tensor.matmul(ps[:], mm(w_l0w[:, 0:512][:, tsl]),
                                     mm(x2Ta[:, cs]), start=True, stop=False)
                    nc.tensor.matmul(ps[:], mm(w_l0w[:, 512:1024][:, tsl]),
                                     mm(x2Tb[:, cs]), start=False, stop=True)
                    nc.vector.tensor_reduce(pool4[:, c:c + 1], ps[:, :],
                                            axis=AX.X, op=OP.max)
                pool1 = cp.tile([128, 1], f32, name="pool1")
                nc.vector.tensor_reduce(pool1[:, :], pool4[:, :], axis=AX.X, op=OP.max)
                nc.scalar.activation(pooled[:, t_:t_ + 1], pool1[:, :],
                                     ACT.Relu, bias=b_l0b[:, t_:t_ + 1])
            y1 = g.tile([128, 2], f32, name="y1")
            for h in range(2):
                ps1 = cpp.tile([128, 1], f32, name="ps_l1", tag="ps_s")
                for c in range(4):
                    nc.tensor.matmul(ps1[:],
                                     w_l1w[:, c * 256 + h * 128: c * 256 + (h + 1) * 128].bitcast(f32),
                                     pooled[:, c:c + 1],
                                     start=(c == 0), stop=(c == 3))
                nc.scalar.activation(y1[:, h:h + 1], ps1[:, :], ACT.Relu,
                                     bias=b_l1b[:, h:h + 1])
            y2 = g.tile([128, 2], f32, name="y2")
            for h in range(2):
                ps2_ = cpp.tile([128, 1], f32, name="ps_l2", tag="ps_s")
                for c in range(2):
                    nc.tensor.matmul(ps2_[:],
                                     w_l2w[:, c * 256 + h * 128: c * 256 + (h + 1) * 128].bitcast(f32),
                                     y1[:, c:c + 1],
                                     start=(c == 0), stop=(c == 1))
                nc.scalar.activation(y2[:, h:h + 1], ps2_[:, :], ACT.Relu,
                                     bias=b_l2b[:, h:h + 1])
            ps3 = cpp.tile([NUM_CLASSES, 1], f32, name="ps_l3", tag="ps_s")
            for c in range(2):
                nc.tensor.matmul(ps3[:],
                                 w_l3w[:, c * NUM_CLASSES:(c + 1) * NUM_CLASSES].bitcast(f32),
                                 y2[:, c:c + 1],
                                 start=(c == 0), stop=(c == 1))
            y3 = cp.tile([NUM_CLASSES, 1], f32, name="y3")
            nc.vector.tensor_add(y3[:, :], ps3[:, :], b_l3b[:, :])
            pr = cpp.tile([1, NUM_CLASSES], f32, name="pr", tag="ps_s")
            nc.tensor.transpose(pr[:], y3[:, :], ident_sb[0:NUM_CLASSES, 0:NUM_CLASSES])
            row = cp.tile([1, NUM_CLASSES], f32, name="row")
            nc.vector.tensor_copy(row[:, :], pr[:, :])
            mx = cp.tile([1, 1], f32, name="mx")
            nc.vector.tensor_reduce(mx[:, :], row[:, :], axis=AX.X, op=OP.max)
            nmx = cp.tile([1, 1], f32, name="nmx")
            nc.scalar.mul(nmx[:, :], mx[:, :], -1.0)
            ex = cp.tile([1, NUM_CLASSES], f32, name="ex")
            sacc = cp.tile([1, 1], f32, name="sacc")
            nc.scalar.activation(ex[:, :], row[:, :], ACT.Exp,
                                 bias=nmx[:, 0:1], accum_out=sacc[:, :])
            lnz = cp.tile([1, 1], f32, name="lnz")
            nc.scalar.activation(lnz[:, :], sacc[:, :], ACT.Ln)
            shift = cp.tile([1, 1], f32, name="shift")
            nc.vector.tensor_sub(shift[:, :], lnz[:, :], nmx[:, :])
            osb = cp.tile([1, NUM_CLASSES], f32, name="osb")
            nc.vector.tensor_scalar(osb[:, :], row[:, :], shift[:, 0:1],
                                    None, op0=OP.subtract)
            nc.sync.dma_start(out[:, :], osb[:, :])

        ctx.close()

    nc.compile()
    return nc


def _get_program():
    if "nc" not in _PROGRAM_CACHE:
        _PROGRAM_CACHE["nc"] = _build_program()
    return _PROGRAM_CACHE["nc"]


def _in_maps(inputs):
    w_names = ["c1w1", "c1b1", "c1w2", "c1b2", "c1w3", "c1b3",
               "c2w1", "c2b1", "c2w2", "c2b2", "c2w3", "c2b3",
               "l0w", "l0b", "l1w", "l1b", "l2w", "l2b", "l3w", "l3b"]
    shared = {k: np.ascontiguousarray(np.asarray(inputs[k], np.float32))
              for k in w_names}
    shared["ident"] = np.eye(128, dtype=np.float32)
    pos = np.ascontiguousarray(np.asarray(inputs["pos"], np.float32))
    maps = []
    for c in range(NCLOUD):
        m = dict(shared)
        m["pos"] = np.ascontiguousarray(pos[c * N:(c + 1) * N])
        maps.append(m)
    return maps


def kernel(**inputs) -> np.ndarray:
    from concourse import bass_utils
    nc = _get_program()
    maps = _in_maps(inputs)
    res = bass_utils.run_bass_kernel_spmd(nc, maps, core_ids=list(range(NCLOUD)))
    outs = [np.asarray(r["out"]).reshape(1, NUM_CLASSES) for r in res.results]
    return np.concatenate(outs, axis=0).astype(np.float32)


# revision 35
# speedup vs baseline: 1.4218x; 1.4218x over previous
"""DGCNN (dynamic edge conv x2 + classifier) Trainium2 Bass kernel.

Sharding: data-parallel over the 8 point clouds -> 8 NeuronCores.

v2 pipeline (per 128-point tile):
  * kNN scores via one augmented matmul (as v1): psc = [x,1]^T [-2x; |x|^2],
    ACT negate-copy (-psc - s_i) -> negS fp32 in SBUF.
  * top-24 via hierarchical DVE selection: 8 group max8/max_index over
    256-wide column groups -> 64 exact candidates; candidates packed into
    fp32 keys (12-bit quantized value * 2048 + (2047 - global_idx)) so the
    final top-24 merge needs only max8/match_replace and the winning
    indices fall out of the key bits with 3 tiny tensor_scalar ops.
  * neighbor features fetched with ONE gpsimd dma_gather(transpose=True)
    per tile straight from DRAM v-rows (bf16, 256B each) into channel-major
    [128ch, 20*128] SBUF layout - no PE transposes, no per-slot DMAs.
  * z1 = relu(a_i + v_j) as a single bf16 DVE add (2x mode) + ACT relu.
  * edge-MLP layers 2/3 in bf16; layer-3 output max-reduced over K=20
    straight out of 2-bank PSUM tiles (k-contiguous 320-column blocks).
"""

import os
import sys
import numpy as np

for _p in ("/opt/trn_rl_repo",):
    if _p not in sys.path:
        sys.path.insert(0, _p)

N = 2048          # points per cloud
NCLOUD = 8
P = 128           # partition tile
NT = N // P       # 16 row tiles
KNN = 20
KSEL = 24         # 3 rounds x 8
G = 4             # score groups per row (hierarchical top-k)
GW = N // G       # 512 group width
NCAND = G * 8     # 32 merge candidates
NEG_BIG = -3.0e38
NUM_CLASSES = 40
F23 = 8388608.0   # 2^23
RNGQ = 128.0      # d2 quantization range for the candidate merge keys
# 11-bit quantized value in a x4096 granule; quantization happens via fp32
# ulp rounding at 2^35 magnitude (single tensor_scalar), index payload
# 2048-jg (never 0, so the floor-extraction add at 2^35-2048 has no
# round-half tie).
SQ4 = (2047.0 / RNGQ) * 4096.0
B35 = (2047.0 + F23) * 4096.0    # 2^35 + 2^23 - 2^12, exactly representable
T35 = 34359738368.0              # 2^35
W2B = T35 - 2048.0               # floor-extraction bias

MM_FAST_MLP = True   # f32r fast path for classifier matmuls

DEBUG = os.environ.get("BASSK_DEBUG") == "1"

_PROGRAM_CACHE = {}


def _build_program():
    import concourse.bass as bass
    import concourse.bacc as bacc
    import concourse.tile as tile
    from concourse import mybir

    f32 = mybir.dt.float32
    f32r = mybir.dt.float32r
    bf16 = mybir.dt.bfloat16
    u16 = mybir.dt.uint16
    i16 = mybir.dt.int16
    AX = mybir.AxisListType
    OP = mybir.AluOpType
    ACT = mybir.ActivationFunctionType

    def mm(ap):
        return ap.bitcast(f32r) if MM_FAST_MLP else ap

    mmo = mm

    nc = bacc.Bacc("TRN2", target_bir_lowering=False, debug=False,
                   num_swdge_queues=4)

    # ---------------- I/O ----------------
    def din(name, shape):
        return nc.dram_tensor(name, list(shape), f32, kind="ExternalInput").ap()

    pos = din("pos", [N, 3])
    c1w1 = din("c1w1", [6, 64]);   c1b1 = din("c1b1", [64])
    c1w2 = din("c1w2", [64, 64]);  c1b2 = din("c1b2", [64])
    c1w3 = din("c1w3", [64, 64]);  c1b3 = din("c1b3", [64])
    c2w1 = din("c2w1", [128, 128]); c2b1 = din("c2b1", [128])
    c2w2 = din("c2w2", [128, 128]); c2b2 = din("c2b2", [128])
    c2w3 = din("c2w3", [128, 256]); c2b3 = din("c2b3", [256])
    l0w = din("l0w", [256, 512]);  l0b = din("l0b", [512])
    l1w = din("l1w", [512, 256]);  l1b = din("l1b", [256])
    l2w = din("l2w", [256, 256]);  l2b = din("l2b", [256])
    l3w = din("l3w", [256, NUM_CLASSES]); l3b = din("l3b", [NUM_CLASSES])
    ident = din("ident", [128, 128])

    out = nc.dram_tensor("out", [1, NUM_CLASSES], f32, kind="ExternalOutput").ap()

    dbg = {}
    if DEBUG:
        for nm, shp, dt_ in [
            ("d_negS0", [128, N], f32), ("d_candv0", [128, NCAND], f32),
            ("d_key0", [128, NCAND], f32), ("d_j24_0", [128, KSEL], f32),
            ("d_idxw0", [128, 8 * KNN], f32),
            ("d_gath0", [128, KNN * 128], f32),
            ("d_z1T0", [128, KNN * 128], f32),
            ("d_x1T", [64, N], f32),
            ("d_x2Ta", [128, N], f32), ("d_x2Tb", [128, N], f32),
        ]:
            dbg[nm] = nc.dram_tensor(nm, shp, dt_, kind="ExternalOutput").ap()

    with tile.TileContext(nc) as tc:
        from contextlib import ExitStack

        ctx = ExitStack()
        g = ctx.enter_context(tc.tile_pool(name="g", bufs=1))          # persistent
        dpool = ctx.enter_context(tc.tile_pool(name="dram", bufs=1, space="DRAM"))

        ident_sb = g.tile([128, 128], f32)
        nc.sync.dma_start(ident_sb[:], ident[:, :])

        A1 = g.tile([3, N], f32r)      # x^T (f32r for fast score matmuls)
        B1 = g.tile([3, N], f32r)      # -2 x^T
        s1r = g.tile([1, N], f32r)     # |x|^2 row
        A2 = g.tile([64, N], f32r)     # x1^T
        B2 = g.tile([64, N], f32r)     # -2 x1^T
        s2r = g.tile([1, N], f32r)     # |x1|^2 row
        onesr = g.tile([1, N], f32r)   # ones row (rank-1 score lhsT)
        a1T = g.tile([64, N], f32)     # u1 - v1 + b1  (channel-major)
        a1Tb = g.tile([64, N], bf16)
        a2T = g.tile([128, N], bf16)
        x2Ta = g.tile([128, N], f32)   # conv2 out ch 0:128
        x2Tb = g.tile([128, N], f32)   # conv2 out ch 128:256
        nscol1 = g.tile([128, NT], f32)  # -s_i per tile column
        nscol2 = g.tile([128, NT], f32)
        vscratch = g.tile([128, N], f32, name="vscratch")

        # candidate-key merge constant: col j -> 2048 - GW*(j//8)
        C32 = g.tile([128, NCAND], f32, name="C32")
        for gg in range(G):
            nc.vector.memset(C32[:, gg * 8:(gg + 1) * 8], 2048.0 - GW * gg)

        v1d = dpool.tile([N, 128], bf16, name="v1d")
        v2d = dpool.tile([N, 128], bf16, name="v2d")

        wraw = ctx.enter_context(tc.tile_pool(name="wraw", bufs=2))

        def load_w(name, shape, pieces, wdt=f32):
            if wdt != f32:
                raw = wraw.tile(list(shape), f32, name=name + "_raw", tag="wraw")
                for sl, srcap in pieces:
                    nc.sync.dma_start(raw[sl], srcap)
                t = g.tile(list(shape), wdt, name=name)
                nc.scalar.copy(t[:, :], raw[:, :])
            else:
                t = g.tile(list(shape), f32, name=name)
                for sl, srcap in pieces:
                    nc.sync.dma_start(t[sl], srcap)
            return t

        SALL = (slice(None), slice(None))
        w_c1w1a = load_w("w_c1w1a", [3, 64], [(SALL, c1w1[0:3, :])], wdt=f32r)
        w_c1w1b = load_w("w_c1w1b", [3, 64], [(SALL, c1w1[3:6, :])], wdt=f32r)
        w_c2w1a = load_w("w_c2w1a", [64, 128], [(SALL, c2w1[0:64, :])], wdt=f32r)
        w_c2w1b = load_w("w_c2w1b", [64, 128], [(SALL, c2w1[64:128, :])], wdt=f32r)
        w_c1w2 = load_w("w_c1w2", [64, 64], [(SALL, c1w2[:, :])], wdt=bf16)
        w_c1w3 = load_w("w_c1w3", [64, 64], [(SALL, c1w3[:, :])], wdt=bf16)
        w_c2w2 = load_w("w_c2w2", [128, 128], [(SALL, c2w2[:, :])], wdt=bf16)
        w_c2w3 = load_w("w_c2w3", [128, 256], [(SALL, c2w3[:, :])], wdt=bf16)
        wdt_head = f32r if MM_FAST_MLP else f32
        w_l0w = load_w("w_l0w", [128, 1024],
                       [((slice(None), slice(0, 512)), l0w[0:128, :]),
                        ((slice(None), slice(512, 1024)), l0w[128:256, :])],
                       wdt=wdt_head)
        w_l1w = load_w("w_l1w", [128, 1024],
                       [((slice(None), slice(c * 256, (c + 1) * 256)),
                         l1w[c * 128:(c + 1) * 128, :]) for c in range(4)],
                       wdt=wdt_head)
        w_l2w = load_w("w_l2w", [128, 512],
                       [((slice(None), slice(0, 256)), l2w[0:128, :]),
                        ((slice(None), slice(256, 512)), l2w[128:256, :])],
                       wdt=wdt_head)
        w_l3w = load_w("w_l3w", [128, 2 * NUM_CLASSES],
                       [((slice(None), slice(0, NUM_CLASSES)), l3w[0:128, :]),
                        ((slice(None), slice(NUM_CLASSES, 2 * NUM_CLASSES)), l3w[128:256, :])],
                       wdt=wdt_head)

        def col(name, src, n):
            t = g.tile([n, 1], f32, name=name)
            nc.sync.dma_start(t[:, :], src.rearrange("(c o) -> c o", o=1))
            return t

        b_c1b1 = col("b_c1b1", c1b1, 64)
        b_c1b2 = col("b_c1b2", c1b2, 64)
        b_c1b3 = col("b_c1b3", c1b3, 64)
        b_c2b1 = col("b_c2b1", c2b1, 128)
        b_c2b2 = col("b_c2b2", c2b2, 128)
        b_c2b3 = g.tile([128, 2], f32)
        nc.sync.dma_start(b_c2b3[:, 0:1], c2b3.rearrange("(h c o) -> h c o", h=2, o=1)[0])
        nc.sync.dma_start(b_c2b3[:, 1:2], c2b3.rearrange("(h c o) -> h c o", h=2, o=1)[1])
        b_l0b = g.tile([128, 4], f32)
        for t_ in range(4):
            nc.sync.dma_start(b_l0b[:, t_:t_ + 1],
                              l0b.rearrange("(h c o) -> h c o", h=4, o=1)[t_])
        b_l1b = g.tile([128, 2], f32)
        for t_ in range(2):
            nc.sync.dma_start(b_l1b[:, t_:t_ + 1],
                              l1b.rearrange("(h c o) -> h c o", h=2, o=1)[t_])
        b_l2b = g.tile([128, 2], f32)
        for t_ in range(2):
            nc.sync.dma_start(b_l2b[:, t_:t_ + 1],
                              l2b.rearrange("(h c o) -> h c o", h=2, o=1)[t_])
        b_l3b = col("b_l3b", l3b, NUM_CLASSES)

        # ones row (f32r) for the rank-1 |x_j|^2 score matmul
        ones_f = g.tile([1, N], f32, name="ones_f")
        nc.vector.memset(ones_f[:, :], 1.0)
        nc.scalar.copy(onesr[:, :], ones_f[:, :])

        # =============== conv1 prep ===============
        with tc.tile_pool(name="prep", bufs=2) as pp, \
             tc.tile_pool(name="prep_ps", bufs=2, space="PSUM") as ppp:
            scol = g.tile([128, NT], f32, name="scol1_pos")
            for i in range(NT):
                isl = slice(i * P, (i + 1) * P)
                pt = pp.tile([128, 3], f32, name="pt")
                nc.sync.dma_start(pt[:], pos[isl, :])
                sq = pp.tile([128, 3], f32, name="sq")
                nc.scalar.activation(sq[:], pt[:], ACT.Square,
                                     accum_out=scol[:, i:i + 1])
                tp = ppp.tile([3, 128], f32, name="tp", space="PSUM", bufs=1)
                nc.tensor.transpose(tp[:], pt[:], ident_sb[:])
                nc.scalar.copy(A1[0:3, isl], tp[:])
            nc.scalar.mul(nscol1[:, :], scol[:, :], -1.0)
            nc.scalar.mul(B1[0:3, :], A1[0:3, :].bitcast(f32), -2.0)
            stp = ppp.tile([NT, 128], f32, name="stp", space="PSUM", bufs=1)
            nc.tensor.transpose(stp[:], scol[:, :], ident_sb[:])
            srow_sb = pp.tile([NT, 128], f32, name="srow_sb")
            nc.scalar.copy(srow_sb[:, :], stp[:, :])
            s1f = pp.tile([1, N], f32, name="s1f")
            nc.sync.dma_start(
                s1f[:, :].rearrange("o (p n) -> o p n", p=NT), srow_sb[:, :])
            nc.scalar.copy(s1r[:, :], s1f[:, :])

            # u1/v1, a1T, v1 rows -> DRAM (bf16, padded to 128 ch = 256B rows)
            for c in range(4):
                cs = slice(c * 512, (c + 1) * 512)
                pu = ppp.tile([64, 512], f32, name="pu", space="PSUM")
                nc.tensor.matmul(pu[:], w_c1w1a[:, :], A1[0:3, cs])
                nc.scalar.activation(a1T[:, cs], pu[:], ACT.Identity, bias=b_c1b1[:, 0:1])
                pv = ppp.tile([64, 512], f32, name="pv", space="PSUM")
                nc.tensor.matmul(pv[:], w_c1w1b[:, :], A1[0:3, cs])
                nc.scalar.copy(vscratch[0:64, cs], pv[:])
                nc.vector.tensor_sub(a1T[:, cs], a1T[:, cs], pv[:])
            nc.scalar.copy(a1Tb[:, :], a1T[:, :])
            for grp in range(4):
                vstage = pp.tile([128, 512], bf16, name="vstage")
                for m in range(4):
                    i = grp * 4 + m
                    tvp = ppp.tile([128, 64], f32, name="tvp", space="PSUM")
                    nc.tensor.transpose(tvp[:], vscratch[0:64, i * P:(i + 1) * P],
                                        ident_sb[0:64, 0:64])
                    nc.vector.memset(vstage[:, m * 128 + 64:(m + 1) * 128], 0.0)
                    nc.vector.tensor_copy(vstage[:, m * 128:m * 128 + 64], tvp[:])
                nc.sync.dma_start(
                    v1d[:, :].rearrange("(g m r) ch -> g r m ch", g=4, m=4)[grp],
                    vstage[:, :])

        # =============== edge-conv block (shared structure) ===============
        gsem = [nc.alloc_semaphore(f"gsem{q}") for q in range(4)]
        gcnt = [0, 0, 0, 0]

        def edge_conv(conv, sp, spp, dsp):
            if conv == 1:
                H, CON = 64, 3
                Asb, Bsb, srow, aTb, vd, nscol = A1, B1, s1r, a1Tb, v1d, nscol1
                wl2, wl3 = w_c1w2, w_c1w3
                bl2, bl3 = b_c1b2, b_c1b3
                nhalf = 1
            else:
                H, CON = 128, 64
                Asb, Bsb, srow, aTb, vd, nscol = A2, B2, s2r, a2T, v2d, nscol2
                wl2, wl3 = w_c2w2, w_c2w3
                bl2, bl3 = b_c2b2, b_c2b3
                nhalf = 2

            state = {}

            def stage_scores(i):
                isl = slice(i * P, (i + 1) * P)
                negS = sp.tile([128, N], f32, name="negS", tag="negS", bufs=3)
                for c in range(4):
                    cs = slice(c * 512, (c + 1) * 512)
                    psc = spp.tile([128, 512], f32, name="psc", tag="psc", bufs=2)
                    nc.tensor.matmul(psc[:, :], Asb[0:CON, isl], Bsb[0:CON, cs],
                                     start=True, stop=False)
                    nc.tensor.matmul(psc[:, :], onesr[0:1, isl], srow[0:1, cs],
                                     start=False, stop=True)
                    nc.scalar.activation(negS[:, cs], psc[:, :], ACT.Identity,
                                         bias=nscol[:, i:i + 1], scale=-1.0)
                state[i] = {"negS": negS}
                if DEBUG and i == 0 and conv == 1:
                    nc.sync.dma_start(dbg["d_negS0"], negS[:, :])

            def stage_topk(i):
                negS = state[i]["negS"]
                # hierarchical selection: per-group exact top-8 (values+indices)
                candv = sp.tile([128, NCAND], f32, name="candv", tag="candv", bufs=2)
                candi = sp.tile([128, NCAND], u16, name="candi", tag="candi", bufs=2)
                for gg in range(G):
                    nc.vector.max(candv[:, gg * 8:(gg + 1) * 8],
                                  negS[:, gg * GW:(gg + 1) * GW])
                for gg in range(G):
                    nc.vector.max_index(candi[:, gg * 8:(gg + 1) * 8],
                                        candv[:, gg * 8:(gg + 1) * 8],
                                        negS[:, gg * GW:(gg + 1) * GW])
                # pack: key = q11*4096 + 2048 - global_idx, via one ulp-rounded
                # quantize at 2^35 plus a fused un-bias/add chain
                tq = sp.tile([128, NCAND], f32, name="tq", tag="tq")
                nc.vector.tensor_scalar(tq[:, :], candv[:, :], SQ4, B35,
                                        op0=OP.mult, op1=OP.add)
                cif = sp.tile([128, NCAND], f32, name="cif", tag="cif")
                nc.vector.tensor_scalar(cif[:, :], candi[:, :], -1.0, None,
                                        op0=OP.mult)
                yq = sp.tile([128, NCAND], f32, name="yq", tag="yq")
                nc.vector.scalar_tensor_tensor(yq[:, :], tq[:, :], T35, cif[:, :],
                                               op0=OP.subtract, op1=OP.add)
                key = sp.tile([128, NCAND], f32, name="key", tag="key")
                nc.vector.tensor_add(key[:, :], yq[:, :], C32[:, :])
                if DEBUG and i == 0 and conv == 1:
                    nc.sync.dma_start(dbg["d_candv0"], candv[:, :])
                    nc.sync.dma_start(dbg["d_key0"], key[:, :])
                # top-24 merge on packed keys
                vals = sp.tile([128, KSEL], f32, name="vals", tag="vals")
                for r in range(3):
                    rs = slice(r * 8, (r + 1) * 8)
                    nc.vector.max(vals[:, rs], key[:, :])
                    if r < 2:
                        nc.vector.match_replace(key[:, :], vals[:, rs],
                                                key[:, :], NEG_BIG)
                # index extraction from key bits (floor at 2^35 ulp, no ties)
                ut = sp.tile([128, KSEL], f32, name="ut", tag="ut")
                nc.vector.tensor_scalar(ut[:, :], vals[:, :], W2B, T35,
                                        op0=OP.add, op1=OP.subtract)
                j24 = sp.tile([128, KSEL], i16, name="j24", tag="j24")
                nc.vector.scalar_tensor_tensor(j24[:, :], ut[:, :], 2048.0,
                                               vals[:, :], op0=OP.add,
                                               op1=OP.subtract)
                if DEBUG and i == 0 and conv == 1:
                    nc.gpsimd.dma_start(dbg["d_j24_0"], j24[:, :])
                # restride to the wrapped int16 layout dma_gather expects:
                # flat slot s = (n_hi*20 + k)*16 + p  ->  idxw[p, n_hi*20+k],
                # replicated into all 8 16-partition groups.  SBUF APs cannot
                # regroup partitions inside free dims, so the fold bounces
                # through flat DRAM (j24 -> jd -> jw wrapped -> broadcast).
                jd = dsp.tile([128, KSEL], i16, name="jd", tag="jd", bufs=2)
                jw = dsp.tile([16, 8 * KNN], i16, name="jw", tag="jw", bufs=2)
                jw2 = dsp.tile([128, 8 * KNN], i16, name="jw2", tag="jw2", bufs=2)
                idxw = sp.tile([128, 8 * KNN], i16, name="idxw", tag="idxw", bufs=3)
                nc.sync.dma_start(jd[:, :], j24[:, :])
                nc.sync.dma_start(
                    jw[:, :].rearrange("p (h k) -> p h k", k=KNN),
                    jd.rearrange("(h p) k -> p h k", p=16)[:, :, 0:KNN])
                nc.scalar.dma_start(
                    jw2[:, :].rearrange("(g p) c -> g p c", p=16),
                    jw[:, :].rearrange("p (o c) -> o p c", o=1)
                        .to_broadcast([8, 16, 8 * KNN]))
                nc.scalar.dma_start(idxw[:, :], jw2[:, :])
                # channel-major gather of the 20*128 neighbor rows; split into
                # 640-idx chunks (a single 2560-idx transpose gather wedges
                # the device - descriptor-ring scale limit).
                gathT = sp.tile([128, KNN * 128], bf16, name="gath", tag="gath",
                                bufs=3)
                for it in range(4):
                    nc.gpsimd.dma_gather(
                        gathT[:, it * 640:(it + 1) * 640]
                            .rearrange("p (o n) -> p o n", o=1),
                        vd[:, :], idxw[:, it * 40:(it + 1) * 40],
                        num_idxs=640, num_idxs_reg=640,
                        elem_size=128, transpose=True,
                        queue_num=it).then_inc(gsem[it], 16)
                    gcnt[it] += 16
                # Tile's SWDGE completion lanes are not queue-aware; a later
                # gather on another queue can bump the lane a consumer waits
                # on.  Gate consumers on explicit per-queue counts instead.
                state[i].update(gathT=gathT, gtarget=tuple(gcnt))
                if DEBUG and i == 0 and conv == 1:
                    nc.gpsimd.dma_start(dbg["d_idxw0"], idxw[:, :])
                    nc.gpsimd.dma_start(dbg["d_gath0"], gathT[:, :])

            def stage_mlp(i):
                isl = slice(i * P, (i + 1) * P)
                gathT = state[i]["gathT"]
                for q in range(4):
                    nc.vector.wait_ge(gsem[q], state[i]["gtarget"][q])
                # z1 = relu(a_i + v_j), channel-major, bf16
                z1T = sp.tile([H, KNN * 128], bf16, name="z1T", tag="z1T", bufs=2)
                gv = gathT[0:H, :].rearrange("c (a k p) -> c a k p", k=KNN, p=16)
                av = aTb[:, isl].rearrange("c (a o p) -> c a o p", o=1, p=16) \
                                .to_broadcast([H, 8, KNN, 16])
                nc.vector.tensor_add(
                    z1T.rearrange("c (a k p) -> c a k p", k=KNN, p=16), gv, av)
                nc.scalar.activation(z1T[:, :], z1T[:, :], ACT.Relu)
                if DEBUG and i == 0 and conv == 1:
                    nc.gpsimd.dma_start(dbg["d_z1T0"][0:H, :], z1T[:, :])
                # ---- layer 2 ----
                z2T = sp.tile([H, KNN * 128], bf16, name="z2T", tag="z2T", bufs=2)
                for c in range(5):
                    cs = slice(c * 512, (c + 1) * 512)
                    pm = spp.tile([H, 512], f32, name="pm", tag="pm", bufs=2)
                    nc.tensor.matmul(pm[:], wl2[:, :], z1T[:, cs])
                    nc.scalar.activation(z2T[:, cs], pm[:], ACT.Relu,
                                         bias=bl2[:, 0:1])
                # ---- layer 3 + max over K (k-contiguous 320-col blocks) ----
                red = sp.tile([128, 128], f32, name="red", tag="red", bufs=2)
                for h in range(nhalf):
                    wsel = wl3[:, :] if conv == 1 else wl3[:, h * 128:(h + 1) * 128]
                    for t2 in range(4):
                        pl = spp.tile([H, 1024], f32, name="pl", tag="pl", bufs=2)
                        for b2 in range(2):
                            blk = t2 * 2 + b2
                            nc.tensor.matmul(
                                pl[:, b2 * 512:b2 * 512 + 320], wsel,
                                z2T[:, blk * 320:(blk + 1) * 320])
                        rv = pl.rearrange("c (b r) -> c b r", b=2)[:, :, 0:320] \
                               .rearrange("c b (k p) -> c b p k", p=16)
                        nc.vector.tensor_reduce(
                            red[0:H, t2 * 32:(t2 + 1) * 32]
                                .rearrange("c (b p) -> c b p", b=2),
                            rv, axis=AX.X, op=OP.max)
                    if conv == 1:
                        nc.scalar.activation(A2[0:64, isl], red[0:64, :],
                                             ACT.Relu, bias=bl3[:, 0:1])
                    else:
                        dst = x2Ta if h == 0 else x2Tb
                        nc.scalar.activation(mmo(dst[:, isl]), red[:, :],
                                             ACT.Relu, bias=bl3[:, h:h + 1])
                del state[i]

            stage_scores(0)
            stage_topk(0)
            for i in range(NT):
                if i + 1 < NT:
                    stage_scores(i + 1)
                    stage_topk(i + 1)
                stage_mlp(i)

        # =============== conv1 ===============
        with tc.tile_pool(name="c1", bufs=2) as sp, \
             tc.tile_pool(name="c1d", bufs=2, space="DRAM") as dsp, \
             tc.tile_pool(name="c1ps", bufs=2, space="PSUM") as spp:
            edge_conv(1, sp, spp, dsp)
        if DEBUG:
            nc.sync.dma_start(dbg["d_x1T"], A2[0:64, :].bitcast(f32))

        # =============== conv2 prep ===============
        with tc.tile_pool(name="prep2", bufs=2) as pp, \
             tc.tile_pool(name="prep2_ps", bufs=2, space="PSUM") as ppp:
            nc.scalar.activation(vscratch[0:64, :], A2[0:64, :].bitcast(f32),
                                 ACT.Square)
            ones64 = g.tile([64, 1], f32, name="ones64")
            nc.vector.memset(ones64[:, :], 1.0)
            s2tmp = pp.tile([1, N], f32, name="s2tmp")
            for c in range(4):
                cs = slice(c * 512, (c + 1) * 512)
                ps2 = ppp.tile([1, 512], f32, name="ps2", space="PSUM", bufs=1)
                nc.tensor.matmul(ps2[:], ones64[:, :], vscratch[0:64, cs])
                nc.scalar.copy(s2tmp[0:1, cs], ps2[:])
            nc.scalar.copy(s2r[:, :], s2tmp[:, :])
            for i in range(NT):
                isl = slice(i * P, (i + 1) * P)
                tsc = ppp.tile([128, 1], f32, name="tsc", space="PSUM", bufs=1)
                nc.tensor.transpose(tsc[:], s2tmp[0:1, isl], ident_sb[0:1, 0:1])
                nc.scalar.mul(nscol2[:, i:i + 1], tsc[:], -1.0)
            nc.scalar.mul(B2[0:64, :], A2[0:64, :].bitcast(f32), -2.0)
            for c in range(4):
                cs = slice(c * 512, (c + 1) * 512)
                pu = ppp.tile([128, 512], f32, name="pu2", space="PSUM")
                nc.tensor.matmul(pu[:], w_c2w1a[:, :], A2[0:64, cs])
                nc.scalar.activation(a2T[:, cs], pu[:], ACT.Identity, bias=b_c2b1[:, 0:1])
                pv = ppp.tile([128, 512], f32, name="pv2", space="PSUM")
                nc.tensor.matmul(pv[:], w_c2w1b[:, :], A2[0:64, cs])
                nc.scalar.copy(vscratch[:, cs], pv[:])
                nc.vector.tensor_sub(a2T[:, cs], a2T[:, cs], pv[:])
            for grp in range(4):
                vstage = pp.tile([128, 512], bf16, name="vstage2")
                for m in range(4):
                    i = grp * 4 + m
                    tvp = ppp.tile([128, 128], f32, name="tvp2", space="PSUM")
                    nc.tensor.transpose(tvp[:], vscratch[:, i * P:(i + 1) * P],
                                        ident_sb[:, :])
                    nc.vector.tensor_copy(vstage[:, m * 128:(m + 1) * 128], tvp[:])
                nc.sync.dma_start(
                    v2d[:, :].rearrange("(g m r) ch -> g r m ch", g=4, m=4)[grp],
                    vstage[:, :])

        # =============== conv2 ===============
        with tc.tile_pool(name="c2", bufs=2) as sp, \
             tc.tile_pool(name="c2d", bufs=2, space="DRAM") as dsp, \
             tc.tile_pool(name="c2ps", bufs=2, space="PSUM") as spp:
            edge_conv(2, sp, spp, dsp)

        if DEBUG:
            nc.sync.dma_start(dbg["d_x2Ta"], x2Ta[:, :])
            nc.sync.dma_start(dbg["d_x2Tb"], x2Tb[:, :])

        # =============== classifier ===============
        with tc.tile_pool(name="cls", bufs=2) as cp, \
             tc.tile_pool(name="clsps", bufs=2, space="PSUM") as cpp:
            pooled = g.tile([128, 4], f32, name="pooled")
            for t_ in range(4):
                tsl = slice(t_ * 128, (t_ + 1) * 128)
                pool4 = cp.tile([128, 4], f32, name="pool4")
                for c in range(4):
                    cs = slice(c * 512, (c + 1) * 512)
                    ps = cpp.tile([128, 512], f32, name="ps_l0", tag="ps_l0")
                    nc.tensor.matmul(ps[:], mm(w_l0w[:, 0:512][:, tsl]),
                                     mm(x2Ta[:, cs]), start=True, stop=False)
                    nc.tensor.matmul(ps[:], mm(w_l0w[:, 512:1024][:, tsl]),
                                     mm(x2Tb[:, cs]), start=False, stop=True)
                    nc.vector.tensor_reduce(pool4[:, c:c + 1], ps[:, :],
                                            axis=AX.X, op=OP.max)
                pool1 = cp.tile([128, 1], f32, name="pool1")
                nc.vector.tensor_reduce(pool1[:, :], pool4[:, :], axis=AX.X, op=OP.max)
                nc.scalar.activation(pooled[:, t_:t_ + 1], pool1[:, :],
                                     ACT.Relu, bias=b_l0b[:, t_:t_ + 1])
            y1 = g.tile([128, 2], f32, name="y1")
            for h in range(2):
                ps1 = cpp.tile([128, 1], f32, name="ps_l1", tag="ps_s")
                for c in range(4):
                    nc.tensor.matmul(ps1[:],
                                     w_l1w[:, c * 256 + h * 128: c * 256 + (h + 1) * 128].bitcast(f32),
                                     pooled[:, c:c + 1],
                                     start=(c == 0), stop=(c == 3))
                nc.scalar.activation(y1[:, h:h + 1], ps1[:, :], ACT.Relu,
                                     bias=b_l1b[:, h:h + 1])
            y2 = g.tile([128, 2], f32, name="y2")
            for h in range(2):
                ps2_ = cpp.tile([128, 1], f32, name="ps_l2", tag="ps_s")
                for c in range(2):
                    nc.tensor.matmul(ps2_[:],
                                     w_l2w[:, c * 256 + h * 128: c * 256 + (h + 1) * 128].bitcast(f32),
                                     y1[:, c:c + 1],
                                     start=(c == 0), stop=(c == 1))
                nc.scalar.activation(y2[:, h:h + 1], ps2_[:, :], ACT.Relu,
                                     bias=b_l2b[:, h:h + 1])
            ps3 = cpp.tile([NUM_CLASSES, 1], f32, name="ps_l3", tag="ps_s")
            for c in range(2):
                nc.tensor.matmul(ps3[:],
                                 w_l3w[:, c * NUM_CLASSES:(c + 1) * NUM_CLASSES].bitcast(f32),
                                 y2[:, c:c + 1],
                                 start=(c == 0), stop=(c == 1))
            y3 = cp.tile([NUM_CLASSES, 1], f32, name="y3")
            nc.vector.tensor_add(y3[:, :], ps3[:, :], b_l3b[:, :])
            pr = cpp.tile([1, NUM_CLASSES], f32, name="pr", tag="ps_s")
            nc.tensor.transpose(pr[:], y3[:, :], ident_sb[0:NUM_CLASSES, 0:NUM_CLASSES])
            row = cp.tile([1, NUM_CLASSES], f32, name="row")
            nc.vector.tensor_copy(row[:, :], pr[:, :])
            mx = cp.tile([1, 1], f32, name="mx")
            nc.vector.tensor_reduce(mx[:, :], row[:, :], axis=AX.X, op=OP.max)
            nmx = cp.tile([1, 1], f32, name="nmx")
            nc.scalar.mul(nmx[:, :], mx[:, :], -1.0)
            ex = cp.tile([1, NUM_CLASSES], f32, name="ex")
            sacc = cp.tile([1, 1], f32, name="sacc")
            nc.scalar.activation(ex[:, :], row[:, :], ACT.Exp,
                                 bias=nmx[:, 0:1], accum_out=sacc[:, :])
            lnz = cp.tile([1, 1], f32, name="lnz")
            nc.scalar.activation(lnz[:, :], sacc[:, :], ACT.Ln)
            shift = cp.tile([1, 1], f32, name="shift")
            nc.vector.tensor_sub(shift[:, :], lnz[:, :], nmx[:, :])
            osb = cp.tile([1, NUM_CLASSES], f32, name="osb")
            nc.vector.tensor_scalar(osb[:, :], row[:, :], shift[:, 0:1],
                                    None, op0=OP.subtract)
            nc.sync.dma_start(out[:, :], osb[:, :])

        ctx.close()

    nc.compile()
    return nc


def _get_program():
    if "nc" not in _PROGRAM_CACHE:
        _PROGRAM_CACHE["nc"] = _build_program()
    return _PROGRAM_CACHE["nc"]


def _in_maps(inputs):
    w_names = ["c1w1", "c1b1", "c1w2", "c1b2", "c1w3", "c1b3",
               "c2w1", "c2b1", "c2w2", "c2b2", "c2w3", "c2b3",
               "l0w", "l0b", "l1w", "l1b", "l2w", "l2b", "l3w", "l3b"]
    shared = {k: np.ascontiguousarray(np.asarray(inputs[k], np.float32))
              for k in w_names}
    shared["ident"] = np.eye(128, dtype=np.float32)
    pos = np.ascontiguousarray(np.asarray(inputs["pos"], np.float32))
    maps = []
    for c in range(NCLOUD):
        m = dict(shared)
        m["pos"] = np.ascontiguousarray(pos[c * N:(c + 1) * N])
        maps.append(m)
    return maps


def kernel(**inputs) -> np.ndarray:
    from concourse import bass_utils
    nc = _get_program()
    maps = _in_maps(inputs)
    res = bass_utils.run_bass_kernel_spmd(nc, maps, core_ids=list(range(NCLOUD)))
    outs = [np.asarray(r["out"]).reshape(1, NUM_CLASSES) for r in res.results]
    return np.concatenate(outs, axis=0).astype(np.float32)
